# revision 1
# baseline (speedup 1.0000x reference)
"""Causal GQA self-attention kernel for Trainium2 (8 NeuronCores).

Sharding: 8 cores = batch (2) x kv-head-group (4). Each core computes, for
its (batch b, kv group g): the Q projection for the group's 4 query heads,
K/V projections for its kv head, causal flash attention for those heads,
and the partial output projection (rows of Wo for its heads). The host
sums the 4 partial outputs per batch element.

On-chip layout (per core):
  xT [E=2048, T=2048]   (host-transposed x[b])  -> streamed in 512-col blocks
  qT [d, head, t], kT [d, t] computed via Wq/Wk.T-style matmuls (fp32r)
  v  [t, d] strips from PE-transposing vT
  attention in "S^T orientation": S^T[tk,tq] tiles = kT_strip.T @ qT_chunk,
  exp on ACT (scale folded in), causal mask multiply on DVE (diagonal strips
  only), softmax denominator via DVE adds + one ones[128x128] matmul
  (replicates l to all partitions), Y^T = v_strip.T @ P^T accumulated in
  PSUM, divided by l on DVE. Output projection contracts head dim against
  resident Wo rows.
"""
import math

import numpy as np

import concourse.bass as bass
import concourse.mybir as mybir
from concourse import bacc
from concourse.tile import TileContext
from concourse.masks import make_identity
from concourse.bass_utils import run_bass_kernel_spmd

F32 = mybir.dt.float32
F32R = mybir.dt.float32r

E = 2048          # embedding dim
T = 2048          # sequence length
D = 128           # head dim
G = 4             # query heads per core (= GQA group size)
C = G * D         # 512 projected columns per core
KT = E // 128     # 16 contraction strips
NTCH = T // 512   # 4 t-chunks
SCALE = 1.0 / math.sqrt(D)


def build_attn(repeat: int = 1, phases: int = 3, sps_bufs: int = 3,
               o_bufs: int = 2, p_bufs: int = 6, xblk_bufs: int = 2,
               skip_dve: bool = False, y_bufs: int = 2):
    nc = bacc.Bacc()
    xT = nc.dram_tensor("xT", [E, T], F32R, kind="ExternalInput")
    wq = nc.dram_tensor("wq", [E, C], F32R, kind="ExternalInput")
    wk = nc.dram_tensor("wk", [E, D], F32R, kind="ExternalInput")
    wv = nc.dram_tensor("wv", [E, D], F32R, kind="ExternalInput")
    wo = nc.dram_tensor("wo", [C, E], F32R, kind="ExternalInput")
    mask = nc.dram_tensor("mask", [128, 896], F32, kind="ExternalInput")
    out = nc.dram_tensor("out", [T, E], F32, kind="ExternalOutput")

    xT3 = xT.rearrange("(ko p) t -> p ko t", p=128)
    wq3 = wq.rearrange("(ko p) c -> p ko c", p=128)
    wk3 = wk.rearrange("(ko p) d -> p ko d", p=128)
    wv3 = wv.rearrange("(ko p) d -> p ko d", p=128)
    wo3 = wo.rearrange("(g p) e -> p g e", p=128)

    with TileContext(nc) as tc:
        with tc.tile_pool(name="persist", bufs=1) as persist:
            qT_sb = persist.tile([128, G, T], F32R)
            kT_sb = persist.tile([128, T], F32R)
            v_sb = persist.tile([128, 16, 128], F32R)
            yT_sb = persist.tile([128, G, T], F32R)
            mask_sb = persist.tile([128, 896], F32)
            ones_sb = persist.tile([128, 128], F32)
            ones_r = persist.tile([128, 128], F32R)
            ident_sb = persist.tile([128, 128], F32)
            nc.sync.dma_start(mask_sb[:], mask[:])
            nc.vector.memset(ones_sb[:], 1.0)
            nc.scalar.copy(ones_r[:], ones_sb[:])
            make_identity(nc, ident_sb[:])

            def body():
                # ---------------- Phase 1: projections ----------------
                if phases < 1:
                    nc.sync.dma_start(out[0:128, 0:896], mask_sb[:])
                    return
                with tc.tile_pool(name="wqkv", bufs=1) as wpool, \
                     tc.tile_pool(name="xblk", bufs=xblk_bufs) as xpool, \
                     tc.tile_pool(name="vt", bufs=2) as vtpool, \
                     tc.tile_pool(name="ps1", bufs=6, space="PSUM") as ps1, \
                     tc.tile_pool(name="ps1t", bufs=2, space="PSUM") as ps1t:
                    wq_sb = wpool.tile([128, KT, C], F32R)
                    wk_sb = wpool.tile([128, KT, D], F32R)
                    wv_sb = wpool.tile([128, KT, D], F32R)
                    nc.sync.dma_start(wk_sb[:], wk3)
                    nc.sync.dma_start(wv_sb[:], wv3)
                    for cq in range(G):
                        nc.sync.dma_start(wq_sb[:, :, cq * 128:(cq + 1) * 128],
                                          wq3[:, :, cq * 128:(cq + 1) * 128])

                    for tch in range(NTCH):
                        t0 = tch * 512
                        xblk = xpool.tile([128, KT, 512], F32R)
                        for kq in range(4):
                            nc.sync.dma_start(xblk[:, 4 * kq:4 * kq + 4, :],
                                              xT3[:, 4 * kq:4 * kq + 4, t0:t0 + 512])
                        # kT chunk
                        psk = ps1.tile([128, 512], F32, tag="psp")
                        for k in range(KT):
                            nc.tensor.matmul(psk[:], (wk_sb[:, k, :]),
                                             (xblk[:, k, :]),
                                             start=(k == 0), stop=(k == KT - 1))
                        nc.scalar.copy(kT_sb[:, t0:t0 + 512], psk[:])
                        # vT chunk -> v strips (natural [t, d]) via PE transpose
                        psv = ps1.tile([128, 512], F32, tag="psp")
                        for k in range(KT):
                            nc.tensor.matmul(psv[:], (wv_sb[:, k, :]),
                                             (xblk[:, k, :]),
                                             start=(k == 0), stop=(k == KT - 1))
                        vt_t = vtpool.tile([128, 512], F32)
                        nc.scalar.copy(vt_t[:], psv[:])
                        pst = ps1t.tile([128, 512], F32)
                        for i in range(4):
                            nc.tensor.transpose(pst[:, i * 128:(i + 1) * 128],
                                                vt_t[:, i * 128:(i + 1) * 128],
                                                ident_sb[:])
                        nc.vector.tensor_copy(v_sb[:, tch * 4:(tch + 1) * 4, :],
                                              pst[:])
                        # qT chunks (4 head columns)
                        for c in range(G):
                            psq = ps1.tile([128, 512], F32, tag="psp")
                            for k in range(KT):
                                nc.tensor.matmul(
                                    psq[:], (wq_sb[:, k, c * 128:(c + 1) * 128]),
                                    (xblk[:, k, :]),
                                    start=(k == 0), stop=(k == KT - 1))
                            nc.scalar.copy(qT_sb[:, c, t0:t0 + 512], psq[:])

                if phases < 2:
                    nc.sync.dma_start(out[0:128, 0:T], kT_sb[:].bitcast(F32))
                    nc.sync.dma_start(out[128:256, 0:T], qT_sb[:, 0, :].bitcast(F32))
                    nc.sync.dma_start(out[256:384, 0:T],
                                      v_sb[:].rearrange("p s d -> p (s d)").bitcast(F32))
                    return
                # ---------------- Phase 2: attention ----------------
                with tc.tile_pool(name="wo", bufs=1) as wopool:
                    wo_sb = wopool.tile([128, G, E], F32R)
                    nc.sync.dma_start(wo_sb[:], wo3)

                    if phases == 13:  # debug: skip attention, fill yT from qT
                        for hh in range(G):
                            nc.vector.tensor_copy(yT_sb[:, hh, :], qT_sb[:, hh, :])
                        _skip_attn = True
                    else:
                        _skip_attn = False
                    with tc.tile_pool(name="sb23", bufs=1) as sb23, \
                         tc.tile_pool(name="ps23", bufs=1, space="PSUM") as ps23:
                        for h in ([] if _skip_attn else range(G)):
                            for q in range(NTCH):
                                tq0 = q * 512
                                n_strips = tq0 // 128 + 4
                                la = sb23.tile([128, 512], F32R, tag="lacc", bufs=2)
                                if q != 0:
                                    lb = sb23.tile([128, 512], F32R,
                                                   tag="lacc2", bufs=2)
                                yps = ps23.tile([128, 512], F32, tag="y", bufs=y_bufs)
                                for s in range(n_strips):
                                    o = s - tq0 // 128
                                    # diagonal strips o=1,2: only columns
                                    # [128*o, 512) can be causally valid
                                    j0 = 128 * o if o in (1, 2) else 0
                                    w = 512 - j0
                                    sps = ps23.tile([128, 512], F32, tag="s", bufs=sps_bufs)
                                    nc.tensor.matmul(
                                        sps[:, :w], (kT_sb[:, s * 128:(s + 1) * 128]),
                                        (qT_sb[:, h, tq0 + j0:tq0 + 512]),
                                        start=True, stop=True)
                                    p = sb23.tile([128, 512], F32R, tag="p", bufs=p_bufs)
                                    nc.scalar.activation(
                                        p[:, :w], sps[:, :w],
                                        mybir.ActivationFunctionType.Exp,
                                        scale=SCALE)
                                    if not skip_dve:
                                        if o >= 0:
                                            off = 384 - 128 * o + j0
                                            nc.vector.tensor_mul(
                                                p[:, :w], p[:, :w],
                                                mask_sb[:, off:off + w])
                                        if s == 0:
                                            nc.vector.tensor_copy(la[:], p[:, :512])
                                        elif s == 1 and q == 0:
                                            # chunk 0 strip 1 is narrow: stay on la
                                            nc.vector.tensor_add(
                                                la[:, j0:], la[:, j0:], p[:, :w])
                                        elif s == 1:
                                            nc.gpsimd.tensor_copy(lb[:], p[:, :512])
                                        elif s % 2 == 0 or q == 0:
                                            nc.vector.tensor_add(
                                                la[:, j0:], la[:, j0:], p[:, :w])
                                        else:
                                            nc.gpsimd.tensor_add(
                                                lb[:, j0:], lb[:, j0:], p[:, :w])
                                    nc.tensor.matmul(
                                        yps[:, j0:], (v_sb[:, s, :]), (p[:, :w]),
                                        start=(s == 0), stop=(s == n_strips - 1))
                                # l replicated to all partitions: ones.T @ la
                                if q != 0:
                                    nc.vector.tensor_add(la[:], la[:], lb[:])
                                rps = ps23.tile([128, 512], F32, tag="r", bufs=1)
                                nc.tensor.matmul(rps[:], (ones_r[:]), (la[:]),
                                                 start=True, stop=True)
                                rinv = sb23.tile([128, 512], F32, tag="rinv", bufs=2)
                                nc.vector.reciprocal(rinv[:], rps[:])
                                nc.vector.tensor_mul(yT_sb[:, h, tq0:tq0 + 512],
                                                     yps[:], rinv[:])

                        if phases < 3:
                            nc.sync.dma_start(out[0:128, 0:T], yT_sb[:, 0, :].bitcast(F32))
                            nc.sync.dma_start(out[128:256, 0:T], yT_sb[:, 1, :].bitcast(F32))
                            return
                        # ---------------- Phase 3: output projection ------
                        for tt in range(16):
                            for ech in range(4):
                                e0 = ech * 512
                                pso = ps23.tile([128, 512], F32, tag="o", bufs=o_bufs)
                                for hh in range(G):
                                    nc.tensor.matmul(
                                        pso[:],
                                        (yT_sb[:, hh, tt * 128:(tt + 1) * 128]),
                                        (wo_sb[:, hh, e0:e0 + 512]),
                                        start=(hh == 0), stop=(hh == G - 1))
                                ot = sb23.tile([128, 512], F32, tag="ot", bufs=3)
                                nc.scalar.copy(ot[:], pso[:])
                                nc.sync.dma_start(
                                    out[tt * 128:(tt + 1) * 128, e0:e0 + 512],
                                    ot[:])

            if repeat == 1:
                body()
            else:
                for _rep in range(repeat):
                    if _rep:
                        tc.strict_bb_all_engine_barrier()
                    body()

    nc.compile()
    return nc


def _make_mask():
    r = np.arange(128)[:, None]
    c = np.arange(896)[None, :]
    return (c >= r + 384).astype(np.float32)


_NC = None


def kernel(x, Wq, Wk, Wv, Wo):
    global _NC
    x = np.asarray(x, dtype=np.float32)
    Wq = np.asarray(Wq, dtype=np.float32)
    Wk = np.asarray(Wk, dtype=np.float32)
    Wv = np.asarray(Wv, dtype=np.float32)
    Wo = np.asarray(Wo, dtype=np.float32)
    B = x.shape[0]
    assert x.shape == (B, T, E)

    if _NC is None:
        _NC = build_attn(repeat=1)
    nc = _NC

    xTh = np.ascontiguousarray(np.transpose(x, (0, 2, 1)))
    mask_np = _make_mask()
    in_maps = []
    for core in range(8):
        b, g = divmod(core, 4)
        b = b % B
        in_maps.append({
            "xT": xTh[b],
            "wq": np.ascontiguousarray(Wq[:, g * C:(g + 1) * C]),
            "wk": np.ascontiguousarray(Wk[:, g * D:(g + 1) * D]),
            "wv": np.ascontiguousarray(Wv[:, g * D:(g + 1) * D]),
            "wo": np.ascontiguousarray(Wo[g * C:(g + 1) * C, :]),
            "mask": mask_np,
        })
    res = run_bass_kernel_spmd(nc, in_maps, list(range(8))).results
    outp = np.empty((B, T, E), dtype=np.float32)
    for b in range(B):
        acc = res[4 * b]["out"].astype(np.float64)
        for g in range(1, 4):
            acc += res[4 * b + g]["out"]
        outp[b] = acc.astype(np.float32)
    return outp

